# revision 1
# baseline (speedup 1.0000x reference)
"""WaveNet-style gated residual block (AdvancedSkipResidualBlock) on 8 TRN2 NeuronCores.

Strategy: data-parallel over batch B=8 -> one batch element per NeuronCore.
Per core, the whole block is 7 GEMMs of [512,512] weights x [512, T] activations:
  xc   = x + Wc @ cond + bc                      (1 GEMM,  K=512)
  f    = tanh(W_f0 @ xc(t-2) + W_f1 @ xc(t) + bf)  (2 GEMMs, dilated causal conv K=2, d=2)
  g    = sigmoid(W_g0 @ xc(t-2) + W_g1 @ xc(t) + bg)
  h    = f * g
  res  = Wr @ h + br + xc                        (1 GEMM)
  skip = Ws @ h + bs                             (1 GEMM)
Matmuls run in bf16 (fp32 PSUM accumulation); elementwise/bias/activation fused
onto ScalarE (tanh/sigmoid/identity, one LUT table set) and VectorE
(scalar_tensor_tensor fuses bias-add + residual-add in one op).
Time axis is processed in 8 chunks of 512 with a 2-column halo for the dilated
conv; a 3-stage software pipeline (cond -> conv/gate -> res/skip) keeps PE busy.
"""

import os
import sys
from contextlib import ExitStack

import numpy as np

try:
    import concourse.bass as bass  # noqa: F401
except ImportError:  # pragma: no cover
    sys.path.insert(0, "/opt/trn_rl_repo")
    import concourse.bass as bass  # noqa: F401

import ml_dtypes  # noqa: E402
import concourse.tile as tile  # noqa: E402
from concourse import bacc, mybir  # noqa: E402
from concourse.bass_utils import run_bass_kernel_spmd  # noqa: E402

B, C, T = 8, 512, 4096
P, G = 128, 4          # SBUF partitions, channel groups (C = G*P)
CH, NCH = 512, 8       # time-chunk width, number of chunks (T = NCH*CH)
DIL = 2                # conv dilation (kernel_size=2 -> taps at t-2 and t)

BF16 = mybir.dt.bfloat16
F32 = mybir.dt.float32
AF = mybir.ActivationFunctionType
ALU = mybir.AluOpType

_CACHE: dict = {}


def _build():
    nc = bacc.Bacc("TRN2", target_bir_lowering=False, debug=False, num_devices=B)

    def din(name, shape, dt):
        return nc.dram_tensor(name, shape, dt, kind="ExternalInput").ap()

    def dout(name, shape, dt):
        return nc.dram_tensor(name, shape, dt, kind="ExternalOutput").ap()

    x_d = din("x", [C, T], F32)
    c_d = din("cond", [C, T], BF16)
    wc_d = din("wc", [P, G, G, P], BF16)
    wf_d = din("wf", [P, 2, G, G, P], BF16)
    wg_d = din("wg", [P, 2, G, G, P], BF16)
    wr_d = din("wr", [P, G, G, P], BF16)
    ws_d = din("ws", [P, G, G, P], BF16)
    bc_d = din("bc", [P, G], F32)
    bf_d = din("bf", [P, G], F32)
    bg_d = din("bg", [P, G], F32)
    br_d = din("br", [P, G], F32)
    bs_d = din("bs", [P, G], F32)
    r_d = dout("res", [C, T], F32)
    s_d = dout("skip", [C, T], F32)

    x_r = x_d.rearrange("(g p) t -> p g t", p=P)
    c_r = c_d.rearrange("(g p) t -> p g t", p=P)
    r_r = r_d.rearrange("(g p) t -> p g t", p=P)
    s_r = s_d.rearrange("(g p) t -> p g t", p=P)

    with tile.TileContext(nc) as tc, ExitStack() as ctx:
        const = ctx.enter_context(tc.tile_pool(name="const", bufs=1))
        xin = ctx.enter_context(tc.tile_pool(name="xin", bufs=3))
        cin = ctx.enter_context(tc.tile_pool(name="cin", bufs=3))
        xcp = ctx.enter_context(tc.tile_pool(name="xcp", bufs=4))
        fgp = ctx.enter_context(tc.tile_pool(name="fgp", bufs=4))
        hp = ctx.enter_context(tc.tile_pool(name="hp", bufs=3))
        rop = ctx.enter_context(tc.tile_pool(name="rop", bufs=3))
        sop = ctx.enter_context(tc.tile_pool(name="sop", bufs=3))
        psum = ctx.enter_context(tc.tile_pool(name="psum", bufs=8, space="PSUM"))

        wc_sb = const.tile([P, G, G, P], BF16)
        nc.sync.dma_start(wc_sb[:], wc_d)
        wf_sb = const.tile([P, 2, G, G, P], BF16)
        nc.sync.dma_start(wf_sb[:], wf_d)
        wg_sb = const.tile([P, 2, G, G, P], BF16)
        nc.sync.dma_start(wg_sb[:], wg_d)
        wr_sb = const.tile([P, G, G, P], BF16)
        nc.sync.dma_start(wr_sb[:], wr_d)
        ws_sb = const.tile([P, G, G, P], BF16)
        nc.sync.dma_start(ws_sb[:], ws_d)
        bias_sb = {}
        for name, ap in (("bc", bc_d), ("bf", bf_d), ("bg", bg_d), ("br", br_d), ("bs", bs_d)):
            t = const.tile([P, G], F32, tag=f"b_{name}")
            nc.sync.dma_start(t[:], ap)
            bias_sb[name] = t

        xc_t: dict = {}
        h_t: dict = {}
        for it in range(NCH + 2):
            c0, c1, c2 = it, it - 1, it - 2

            # ---- stage 1: condition injection (chunk c0) ----
            if c0 < NCH:
                lo = c0 * CH
                xt = xin.tile([P, G, CH], F32, tag="x")
                nc.sync.dma_start(xt[:], x_r[:, :, lo:lo + CH])
                ct = cin.tile([P, G, CH], BF16, tag="c")
                nc.sync.dma_start(ct[:], c_r[:, :, lo:lo + CH])
                xc = xcp.tile([P, G, CH + DIL], BF16, tag="xc")
                if c0 == 0:
                    nc.vector.memset(xc[:, :, 0:DIL], 0.0)
                else:
                    nc.vector.tensor_copy(xc[:, :, 0:DIL], xc_t[c0 - 1][:, :, CH:CH + DIL])
                for m in range(G):
                    ps = psum.tile([P, CH], F32, space="PSUM", tag="ps")
                    for k in range(G):
                        nc.tensor.matmul(ps, wc_sb[:, k, m, :], ct[:, k, :],
                                         start=(k == 0), stop=(k == G - 1))
                    # xc[m] = (Wc@cond + bc) + x, cast to bf16
                    nc.vector.scalar_tensor_tensor(
                        xc[:, m, DIL:CH + DIL], ps, bias_sb["bc"][:, m:m + 1],
                        xt[:, m, :], ALU.add, ALU.add)
                xc_t[c0] = xc

            # ---- stage 2: dilated conv + gated activation (chunk c1) ----
            if 0 <= c1 < NCH:
                xc = xc_t[c1]
                h = hp.tile([P, G, CH], BF16, tag="h")
                for m in range(G):
                    pf = psum.tile([P, CH], F32, space="PSUM", tag="ps")
                    for k in range(G):
                        nc.tensor.matmul(pf, wf_sb[:, 0, k, m, :], xc[:, k, 0:CH],
                                         start=(k == 0), stop=False)
                    for k in range(G):
                        nc.tensor.matmul(pf, wf_sb[:, 1, k, m, :], xc[:, k, DIL:CH + DIL],
                                         start=False, stop=(k == G - 1))
                    fsb = fgp.tile([P, CH], BF16, tag="f")
                    nc.scalar.activation(fsb[:], pf, AF.Tanh, bias=bias_sb["bf"][:, m:m + 1])
                    pg = psum.tile([P, CH], F32, space="PSUM", tag="ps")
                    for k in range(G):
                        nc.tensor.matmul(pg, wg_sb[:, 0, k, m, :], xc[:, k, 0:CH],
                                         start=(k == 0), stop=False)
                    for k in range(G):
                        nc.tensor.matmul(pg, wg_sb[:, 1, k, m, :], xc[:, k, DIL:CH + DIL],
                                         start=False, stop=(k == G - 1))
                    gsb = fgp.tile([P, CH], BF16, tag="g")
                    nc.scalar.activation(gsb[:], pg, AF.Sigmoid, bias=bias_sb["bg"][:, m:m + 1])
                    nc.vector.tensor_mul(h[:, m, :], fsb[:], gsb[:])
                h_t[c1] = h

            # ---- stage 3: residual + skip projections (chunk c2) ----
            if 0 <= c2 < NCH:
                h = h_t.pop(c2)
                xc = xc_t.pop(c2)
                lo = c2 * CH
                rt = rop.tile([P, G, CH], F32, tag="r")
                st = sop.tile([P, G, CH], F32, tag="s")
                for m in range(G):
                    pr = psum.tile([P, CH], F32, space="PSUM", tag="ps")
                    for k in range(G):
                        nc.tensor.matmul(pr, wr_sb[:, k, m, :], h[:, k, :],
                                         start=(k == 0), stop=(k == G - 1))
                    # res[m] = (Wr@h + br) + xc
                    nc.vector.scalar_tensor_tensor(
                        rt[:, m, :], pr, bias_sb["br"][:, m:m + 1],
                        xc[:, m, DIL:CH + DIL], ALU.add, ALU.add)
                    pk = psum.tile([P, CH], F32, space="PSUM", tag="ps")
                    for k in range(G):
                        nc.tensor.matmul(pk, ws_sb[:, k, m, :], h[:, k, :],
                                         start=(k == 0), stop=(k == G - 1))
                    nc.scalar.activation(st[:, m, :], pk, AF.Identity,
                                         bias=bias_sb["bs"][:, m:m + 1])
                nc.sync.dma_start(r_r[:, :, lo:lo + CH], rt[:])
                nc.sync.dma_start(s_r[:, :, lo:lo + CH], st[:])

    nc.compile()
    return nc


def _get_nc():
    if "nc" not in _CACHE:
        _CACHE["nc"] = _build()
    return _CACHE["nc"]


def _wT1(w):
    # [Cout, Cin, 1] -> lhsT layout [P(cin%P), G(cin//P), G(cout//P), P(cout%P)]
    return np.ascontiguousarray(
        np.asarray(w)[:, :, 0].T.reshape(G, P, G, P).transpose(1, 0, 2, 3)
        .astype(ml_dtypes.bfloat16))


def _wT2(w):
    # [Cout, Cin, 2] -> [P, tap, G(cin//P), G(cout//P), P]
    taps = [np.asarray(w)[:, :, t].T.reshape(G, P, G, P).transpose(1, 0, 2, 3)
            for t in range(2)]
    return np.ascontiguousarray(np.stack(taps, axis=1).astype(ml_dtypes.bfloat16))


def _bias(b):
    return np.ascontiguousarray(np.asarray(b).reshape(G, P).T.astype(np.float32))


def kernel(x, condition, wf, bf, wg, bg, wr, br, ws, bs, wc, bc):
    nc = _get_nc()
    x = np.asarray(x, dtype=np.float32)
    cond_bf = np.asarray(condition).astype(ml_dtypes.bfloat16)
    shared = {
        "wc": _wT1(wc), "wf": _wT2(wf), "wg": _wT2(wg),
        "wr": _wT1(wr), "ws": _wT1(ws),
        "bc": _bias(bc), "bf": _bias(bf), "bg": _bias(bg),
        "br": _bias(br), "bs": _bias(bs),
    }
    in_maps = [
        {"x": np.ascontiguousarray(x[i]), "cond": np.ascontiguousarray(cond_bf[i]),
         **shared}
        for i in range(B)
    ]
    res = run_bass_kernel_spmd(
        nc, in_maps, list(range(B)),
        trace=bool(os.environ.get("CC_KERNEL_TRACE")))
    _CACHE["last_results"] = res
    residual = np.stack([res.results[i]["res"] for i in range(B)])
    skip = np.stack([res.results[i]["skip"] for i in range(B)])
    return residual, skip
